# revision 15
# baseline (speedup 1.0000x reference)
"""COCOA loss kernel for 8 Trainium2 NeuronCores.

loss = SCALE_LOSS * sum_b pos[b] + LAMBDA * sum(neg)
  pos[b] = mean_{v,w} exp((1 - zn[v,b]·zn[w,b]) / T)           (per-sample view gram)
  neg    = sum_{v,b,c!=b} exp(zn[v,b]·zn[v,c] / T) / (B-1)     (per-view batch gram)

Device strategy (SPMD, one program, per-core data):
  * Host normalizes z, transposes to [V, D, B] bf16 and rolls columns by
    512*core; only local cols [0, 2560) are shipped (each core's row tiles
    need column tiles at circulant distance delta = 0..16 only).
  * Per view the [B, B] gram is covered at 128-row granularity: global row
    tile r (local tile m, r = 4c+m) is multiplied against column tiles at
    delta = 1..15 (host weight 2; the transposed pair comes from the row
    that sees the complementary delta), delta = 0 (diagonal tile) and
    delta = 16 (computed by both paired cores) at effective weight 1 via
    the activation bias: exp(2s - ln2) = exp(2s)/2 with host weight 2.
    This covers every ordered off-diagonal pair exactly once with 13%
    fewer gram elements than the 512-block cover.
  * Gram columns stream into rotating [128, 2048] PSUM units (4 banks x 2);
    ScalarE evaluates exp with the fused free-dim accumulator straight
    from PSUM - one ACTIVATE per unit (the per-instruction overhead, about
    0.4 us, is what made finer-grained variants slow).
  * The true diagonal exp(2 s_bb) ~ e^2 is subtracted analytically on host.
  * pos term: single-pass fused multiply+reduce (tensor_tensor_reduce) on
    VectorE per (tile, view-pair), then two strided exp-accumulate
    ACTIVATEs over all collected sims.
"""

import sys

import numpy as np

try:
    import concourse.bass as bass  # noqa: F401
except ImportError:  # pragma: no cover
    sys.path.insert(0, "/opt/trn_rl_repo")

import concourse.bass as bass
import concourse.bacc as bacc
import concourse.mybir as mybir
import concourse.tile as tile
from concourse.bass_utils import run_bass_kernel_spmd

import ml_dtypes

BF16 = ml_dtypes.bfloat16

# Problem constants (hardcoded per the harness contract).
B = 4096          # batch
V = 6             # views
D = 256           # embedding dim
KC = 2            # contraction chunks of 128 (D = 256)
NCORE = 8
BLK = B // NCORE  # 512 rows per core
MT = BLK // 128   # 4 row tiles of 128 per core
NDELTA = 17       # circulant distances 0..16 per row tile
ZCOLS = 128 * (MT - 1 + NDELTA)  # 2560 local columns shipped per core

TEMPERATURE = 0.5
SCALE_LOSS = 1.0 / 32.0
LAMBDA = 0.0039

UNIT = 2048       # PSUM unit free size (4 banks of fp32)
LN2 = float(np.log(2.0))

F32 = mybir.dt.float32
BF16_DT = mybir.dt.bfloat16

_PAIRS = [(v, w) for v in range(V) for w in range(v + 1, V)]  # 15
_SELF = [(v, v) for v in range(V)]                            # 6


def _neg_units():
    """Pack the gram segments into [128, UNIT] PSUM units.

    Segment = (v, m, col_lo, length): stationary is row tile m of view v,
    moving operand is local columns [col_lo, col_lo+length). Full-weight
    segments (delta 1..15) come first, then the halved class (delta 0 and
    delta 16) in dedicated units so each unit needs a single uniform
    activation bias.

    Returns [(halved, [(off, v, m, col_lo, length), ...]), ...].
    """
    w2_segs = [(v, m, 128 * (m + 1), 128 * (NDELTA - 2))
               for v in range(V) for m in range(MT)]
    half_segs = [(v, m, 128 * m, 128) for v in range(V) for m in range(MT)]
    half_segs += [(v, m, 128 * (m + NDELTA - 1), 128)
                  for v in range(V) for m in range(MT)]

    units = []
    for halved, segs in ((False, w2_segs), (True, half_segs)):
        cur, off = [], 0
        for v, m, lo, ln in segs:
            while ln > 0:
                take = min(ln, UNIT - off)
                cur.append((off, v, m, lo, take))
                off += take
                lo += take
                ln -= take
                if off == UNIT:
                    units.append((halved, cur))
                    cur, off = [], 0
        if cur:
            units.append((halved, cur))
    return units


_UNITS = _neg_units()
NNEG = len(_UNITS)               # 26 (23 full-weight incl. one partial, 3 halved)
NSTAT = NNEG + 2 * MT            # + per-tile pos pairs/selfs cols
POS_COL = NNEG


def _build_nc(reps: int = 1, loop: int = 1) -> bass.Bass:
    """reps/loop > 1 repeat the compute body (inputs stay SBUF-resident) so
    the timing harness can measure steady-state HW time differentially;
    loop uses a hardware For_i around `reps` unrolled bodies."""
    nc = bacc.Bacc("TRN2", debug=False, num_devices=NCORE)

    zt_d = nc.dram_tensor("zt", [V, KC, 128, ZCOLS], BF16_DT, kind="ExternalInput")
    zb_d = nc.dram_tensor("zb", [MT, 128, V * D], BF16_DT, kind="ExternalInput")
    st_d = nc.dram_tensor("stats", [128, NSTAT], F32, kind="ExternalOutput")

    with tile.TileContext(nc) as tc:
        with (
            tc.tile_pool(name="ztp", bufs=1) as ztp,
            tc.tile_pool(name="zbp", bufs=1) as zbp,
            tc.tile_pool(name="stp", bufs=1) as stp,
            tc.tile_pool(name="simsp", bufs=1) as simsp,
            tc.tile_pool(name="prodp", bufs=2) as prodp,
            tc.tile_pool(name="escp", bufs=2) as escp,
            tc.tile_pool(name="psump", bufs=2, space="PSUM") as psump,
        ):
            stats = stp.tile([128, NSTAT], F32)

            # ---- DMA inputs in (column-split to spread across queues) ----
            zt_sb = [[ztp.tile([128, ZCOLS], BF16_DT, tag=f"zt_{v}_{k}",
                               name=f"zt_{v}_{k}")
                      for k in range(KC)] for v in range(V)]
            for v in range(V):
                for k in range(KC):
                    for h in range(2):
                        cs = slice(h * (ZCOLS // 2), (h + 1) * (ZCOLS // 2))
                        nc.sync.dma_start(zt_sb[v][k][:, cs],
                                          zt_d.ap()[v, k][:, cs])
            zb_sb = [zbp.tile([128, V * D], BF16_DT, tag=f"zb_{t}", name=f"zb_{t}")
                     for t in range(MT)]
            for t in range(MT):
                nc.sync.dma_start(zb_sb[t][:, :], zb_d.ap()[t])

            def body():
                run_body(nc, tc, zt_sb, zb_sb, stats, simsp, prodp, escp, psump)

            if loop > 1:
                with tc.For_i(0, loop, 1):
                    for _ in range(reps):
                        body()
            else:
                for _ in range(reps):
                    body()

            # ---- stats out ----
            nc.sync.dma_start(st_d.ap()[:, :], stats[:, :])

    nc.compile()
    return nc


def run_body(nc, tc, zt_sb, zb_sb, stats, simsp, prodp, escp, psump):
    # ---- neg term: circulant gram cover on PE, exp+sum on ACT ----
    for u, (halved, segs) in enumerate(_UNITS):
        used = segs[-1][0] + segs[-1][4]
        ps = psump.tile([128, UNIT], F32, tag="gram", name="gram")
        for off, v, m, lo, ln in segs:
            # chunk on the 512-col PSUM bank grid: a matmul output may not
            # cross a bank boundary
            sub = 0
            while sub < ln:
                w = min(512 - (off + sub) % 512, ln - sub)
                for k in range(KC):
                    nc.tensor.matmul(
                        ps[:, off + sub: off + sub + w],
                        zt_sb[v][k][:, 128 * m: 128 * (m + 1)],
                        zt_sb[v][k][:, lo + sub: lo + sub + w],
                        start=(k == 0),
                        stop=(k == KC - 1),
                    )
                sub += w
        esc = escp.tile([128, UNIT], BF16_DT, tag="esc", name="esc")
        # exp(s / T) = exp(2 s); halved units land in their own stats
        # columns and get host weight 1 instead of 2.
        nc.scalar.activation(
            esc[:, 0:used], ps[:, 0:used],
            mybir.ActivationFunctionType.Exp,
            bias=0.0, scale=2.0,
            accum_out=stats[:, u:u + 1],
        )

    # ---- pos term: fused per-sample cross-view sims on DVE (emitted after
    # the neg phase so its ACT ops sit at the tail of ACT's queue) ----
    sims = simsp.tile([128, MT * 21], F32, tag="sims", name="sims")
    for t in range(MT):
        prods = prodp.tile([128, 21, D], BF16_DT, tag="prods", name="prods")
        for j, (v, w) in enumerate(_PAIRS + _SELF):
            nc.vector.tensor_mul(
                prods[:, j, :],
                zb_sb[t][:, v * D:(v + 1) * D],
                zb_sb[t][:, w * D:(w + 1) * D],
            )
        nc.vector.tensor_reduce(
            sims[:, t * 21:(t + 1) * 21], prods[:, :, :],
            axis=mybir.AxisListType.X, op=mybir.AluOpType.add,
        )
    pexp = escp.tile([128, MT * 21], BF16_DT, tag="pexp", name="pexp")
    # exp((1 - s)/T) = e^2 * exp(-2 s); e^2 applied on host
    for t in range(MT):
        nc.scalar.activation(
            pexp[:, t * 21: t * 21 + 15], sims[:, t * 21: t * 21 + 15],
            mybir.ActivationFunctionType.Exp,
            bias=0.0, scale=-2.0,
            accum_out=stats[:, POS_COL + 2 * t: POS_COL + 2 * t + 1],
        )
        nc.scalar.activation(
            pexp[:, t * 21 + 15: t * 21 + 21], sims[:, t * 21 + 15: t * 21 + 21],
            mybir.ActivationFunctionType.Exp,
            bias=0.0, scale=-2.0,
            accum_out=stats[:, POS_COL + 2 * t + 1: POS_COL + 2 * t + 2],
        )


_NC_CACHE = None


def _get_nc() -> bass.Bass:
    global _NC_CACHE
    if _NC_CACHE is None:
        _NC_CACHE = _build_nc()
    return _NC_CACHE


def _prep_inputs(z: np.ndarray) -> list[dict[str, np.ndarray]]:
    z = np.asarray(z, dtype=np.float32)
    zn = z / np.linalg.norm(z, axis=-1, keepdims=True)          # [B, V, D] f32
    zT = np.ascontiguousarray(zn.transpose(1, 2, 0))            # [V, D, B]
    zt_bf = zT.reshape(V, KC, 128, B).astype(BF16)
    in_maps = []
    for c in range(NCORE):
        zt_c = np.roll(zt_bf, -BLK * c, axis=-1)[..., :ZCOLS]
        zb_c = np.ascontiguousarray(
            zn[BLK * c:BLK * (c + 1)].reshape(MT, 128, V * D)
        ).astype(BF16)
        in_maps.append({"zt": np.ascontiguousarray(zt_c), "zb": zb_c})
    return in_maps


def _host_reduce(stats_list) -> np.float32:
    e2 = float(np.exp(2.0))
    neg_sum = 0.0
    pos_sum = 0.0
    unit_w = np.array([1.0 if halved else 2.0 for halved, _ in _UNITS])
    for c in range(NCORE):
        st = np.asarray(stats_list[c], dtype=np.float64)
        neg_sum += (st[:, 0:NNEG].sum(axis=0) * unit_w).sum()
        pairs = st[:, POS_COL:POS_COL + 2 * MT:2].sum()
        selfs = st[:, POS_COL + 1:POS_COL + 2 * MT + 1:2].sum()
        pos_sum += e2 * (2.0 * pairs + selfs) / (V * V)
    neg_sum -= NCORE * V * BLK * e2            # analytic true-diagonal
    total = SCALE_LOSS * pos_sum + LAMBDA * neg_sum / (B - 1)
    return np.float32(total)


def run(z: np.ndarray, trace: bool = False):
    """Returns (loss, BassKernelResults)."""
    nc = _get_nc()
    in_maps = _prep_inputs(z)
    res = run_bass_kernel_spmd(
        nc, in_maps, core_ids=list(range(NCORE)), trace=trace
    )
    stats_list = [res.results[c]["stats"] for c in range(NCORE)]
    return _host_reduce(stats_list), res


def kernel(z: np.ndarray) -> np.ndarray:
    loss, _ = run(z, trace=False)
    return np.asarray(loss, dtype=np.float32)
